# revision 36
# baseline (speedup 1.0000x reference)
"""Trainium2 Bass kernel for the BDH-style recurrent block.

Strategy: data-parallel over B (8 batches -> 8 NeuronCores, no collectives).
The T=128-step scan is de-sequentialized into dense matmuls per core:

  u_t = relu(emb_t @ Dx.T)                                  (T,N)
  x_t = (XD*x_{t-1} + u_t)/s_t  with s_t = XD + sum(u_t)    (L1 norm; x>=0)
      => x = C @ u, C[t,s] = (1/s_s) exp(A_t - A_s), A_t = cumsum log(XD/s_r)
  a*_t = rho_{t-1} @ x_t = ((DecayMask . X X^T) @ ln(emb))_t   (rho_0 = 0)
  y_t  = relu(ln(a*_t) @ Dy.T) * x_t                        (x_t >= 0)
  v*_t = ln(y_t @ E.T)                                      output

All matmul operands are bf16 (final rel-err ~6e-3 vs the 2e-2 gate).
X^T and Ycore^T are produced directly n-major by taking the time/d axis
as the contraction dim (lhsT=u-chunk / lhsT=DyT-block), removing all PE
transposes. ln(a*) is computed mean-only: lnaT chunks come straight from
matmul(lhsT=vn-chunk, rhs=W^T) plus a rank-1 (ones x -mean) accumulate;
the skipped 1/sqrt(var_a+eps) row scale is folded EXACTLY into the final
layernorm (out = (vraw' - m')/sqrt(v' + eps*(var_a + eps))) since relu is
positively homogeneous and y@E.T is row-linear. The HAM clock gate needs
~3.4us of dense PE activity to lift 1.2->2.4 GHz and re-throttles after
~2us gaps, so junk warmup/keepalive matmuls pad the DMA-wait head and the
serial scalar chains. A single ACT table set (natural_log_exp_and_others)
serves Ln/Exp/Relu/Identity/Copy -> exactly one ACT_TABLE_LOAD.
"""

import math
from contextlib import ExitStack

import numpy as np

N = 2048
D = 256
B = 8
T = 128
XD = 0.97
UD = 0.97
LN_EPS = 1e-5
L1_EPS = 1e-12

# log-domain recentring: E[sum relu(N(0,1)) over 2048] + XD ~ 818.9
LNC2INV = 6.7065
C2 = math.exp(-LNC2INV)
K1 = LNC2INV - math.log(XD)

KD = D // 128   # 2
KN = N // 128   # 16
NJ = N // 512   # 4

# xin bf16 column map: embT | utones | dmaskT | dxT | emb | utones_strict
XI_EMBT = 0
XI_UT = KD * T            # 256
XI_DM = XI_UT + T         # 384
XI_DX = XI_DM + T         # 512
XI_EMB = XI_DX + KD * N   # 4608
XI_UTS = XI_EMB + D       # 4864
XIN_COLS = XI_UTS + T     # 4992
NCF = T + 1               # cf32: trik | xdvec

WARMUP_MMS = 6

_cache = {}
SIM_MODE = False  # CoreSim's xorwow random-fill is broken; use memset there


def _pack_jk(wT):
    # (KD,128,N) k-major -> (128, [j(4), k(2), 512]) per-partition contiguous
    return np.ascontiguousarray(
        wT.reshape(KD, 128, NJ, 512).transpose(1, 2, 0, 3).reshape(128, KD * N))


def _consts_f32():
    r = np.arange(T)
    tri = r[None, :] - r[:, None]                                   # t - s
    trik = np.where(tri >= 0, -K1 * tri - LNC2INV, -10000.0).astype(np.float32)
    xdvec = np.full((T, 1), C2 * XD, dtype=np.float32)
    xdvec[0, 0] = 0.0                                               # x_{-1} = 0
    return np.ascontiguousarray(np.concatenate([trik, xdvec], axis=1))


def _consts_bf():
    r = np.arange(T)
    utones = (r[:, None] <= r[None, :]).astype(np.float32)          # [r,t] r<=t
    uts = (r[:, None] < r[None, :]).astype(np.float32)              # [r,t] r<t
    pw = r[:, None] - 1 - r[None, :]                                # [t,s] t-1-s
    dmask = np.where(pw >= 0, UD ** np.maximum(pw, 0), 0.0).astype(np.float32)
    dmaskT = np.ascontiguousarray(dmask.T)                          # [s,t]
    return utones, dmaskT, uts


def _split_multiwait(nc, mybir):
    """This walrus build caps sync waits per instruction (1 for regular
    instructions, 2 for EventSemaphore). Tile attaches more (e.g. the
    kernel-tail Drain waits on every live semaphore). Hoist excess waits
    onto same-engine NOPs placed immediately before the instruction —
    engine queues are sequential, so semantics are preserved."""
    n = 0
    for f in nc.m.functions:
        for bb in f.blocks:
            out = []
            changed = False
            for ins in bb.instructions:
                si = ins.sync_info
                ow = list(si.on_wait) if si is not None else []
                cap = 2 if ins.opcode == "EventSemaphore" else 1
                if len(ow) > cap:
                    sem_waits = [w for w in ow if w.sync_type == "semaphore"]
                    other = [w for w in ow if w.sync_type != "semaphore"]
                    keep = max(cap - len(other), 0)
                    hoist = sem_waits[:len(sem_waits) - keep] if keep else sem_waits
                    kept = sem_waits[len(hoist):] + other
                    assert len(kept) <= cap, (len(kept), cap, ins.opcode)
                    changed = True
                    for w in hoist:
                        n += 1
                        nop = mybir.InstNoOp(
                            name=f"wsplit-{n}",
                            sync_info=mybir.SyncInfo(on_wait=[w], on_update=[]),
                            bass_nofuse=True,
                            engine=ins.engine,
                        )
                        nc.register_instruction(nop, overwrite=True)
                        out.append(nop)
                    si.on_wait = kept
                out.append(ins)
            if changed:
                bb.instructions = out
    return nc


def _build():
    import concourse.bass as bass
    import concourse.mybir as mybir
    import concourse.tile as tile

    f32 = mybir.dt.float32
    bf16 = mybir.dt.bfloat16
    AF = mybir.ActivationFunctionType
    ALU = mybir.AluOpType
    AX = mybir.AxisListType

    from concourse.vector_clock import ScopedClock

    class _TrimTailTC(tile.TileContext):
        # Drop the second kernel-tail all-engine barrier: it only orders
        # the semaphore resets against engine halt, and nothing executes
        # after it. The first barrier (before resets) is kept, so resets
        # still happen on a quiesced machine and re-execution stays safe.
        def _drain_and_barrier(self, tick_clock, wait_clock):
            drain_inst = self.nc.sync.drain()
            wait_clock.add_sem_waits(
                drain_inst.ins, ScopedClock({None: tick_clock.global_clock})
            )
            self.nc.all_engine_barrier()
            assert self.sems is not None
            popped = self.nc._tile_sem_poison_stack.pop()
            assert popped is self._sem_poison
            self.nc.clear_and_free_semaphores(
                list(self.sems.allocated().values())
            )

    nc = bass.Bass()

    d_xin = nc.dram_tensor("xin", [128, XIN_COLS], bf16, kind="ExternalInput")
    d_cf = nc.dram_tensor("cf", [128, NCF], f32, kind="ExternalInput")
    d_dyT = nc.dram_tensor("dyT", [128, KD * N], bf16, kind="ExternalInput")
    d_eT = nc.dram_tensor("eT", [128, KN * D], bf16, kind="ExternalInput")
    d_out = nc.dram_tensor("out", [T, D], f32, kind="ExternalOutput")

    with _TrimTailTC(nc) as tc, ExitStack() as ctx:
        work = ctx.enter_context(tc.tile_pool(name="work", bufs=1))
        stats = ctx.enter_context(tc.tile_pool(name="stats", bufs=1))
        p_u = ctx.enter_context(tc.tile_pool(name="p_u", bufs=2, space="PSUM"))
        p_nt = ctx.enter_context(tc.tile_pool(name="p_nt", bufs=4, space="PSUM"))
        p_g = ctx.enter_context(tc.tile_pool(name="p_g", bufs=1, space="PSUM"))
        p_med = ctx.enter_context(tc.tile_pool(name="p_med", bufs=1, space="PSUM"))

        # ---- warmup data + tiny consts (no DMA needed) ------------------
        wu_sb = work.tile([128, 512], bf16)
        if SIM_MODE:
            nc.vector.memset(wu_sb[:], 1.0)
        else:
            nc.vector.random(wu_sb[:])
        ones1_sb = work.tile([1, T], bf16)
        nc.vector.memset(ones1_sb[:], 1.0)
        negones_sb = work.tile([1, T], bf16)
        nc.vector.memset(negones_sb[:], -1.0)

        # ---- single ACT table load, forced at t=0: Ln then Exp narrows
        # the possible-set to natural_log_exp_and_others (also has Relu/
        # Identity/Copy) -> exactly one load, off the critical path.
        pre_sb = stats.tile([1, 1], f32)
        nc.vector.memset(pre_sb[:], 1.0)
        pre_o = stats.tile([1, 1], f32)
        nc.scalar.activation(pre_o[:], pre_sb[:], AF.Ln)
        nc.scalar.activation(pre_o[:], pre_sb[:], AF.Exp)

        # ---- DMAs: strict need-order. The head is split across the two
        # HWDGE rings so embT/consts (act ring) and dxT j0 (sync ring)
        # transfer concurrently — each ~2us completion receipt overlaps.
        xin_sb = work.tile([128, XIN_COLS], bf16)
        nc.scalar.dma_start(xin_sb[:, 0:XI_DX], d_xin[:, 0:XI_DX])
        nc.sync.dma_start(xin_sb[:, XI_DX:XI_DX + 1024],
                          d_xin[:, XI_DX:XI_DX + 1024])
        for j in range(1, NJ):
            c0 = XI_DX + j * 1024
            nc.sync.dma_start(xin_sb[:, c0:c0 + 1024], d_xin[:, c0:c0 + 1024])
        nc.sync.dma_start(xin_sb[:, XI_EMB:XIN_COLS], d_xin[:, XI_EMB:XIN_COLS])
        cf_sb = work.tile([128, NCF], f32)
        nc.sync.dma_start(cf_sb[:], d_cf[:])
        dyT_sb = work.tile([128, KD * N], bf16)
        eT_sb = work.tile([128, KN * D], bf16)
        nc.sync.dma_start(dyT_sb[:, 0:2048], d_dyT[:, 0:2048])
        nc.sync.dma_start(eT_sb[:, 0:2048], d_eT[:, 0:2048])
        nc.sync.dma_start(dyT_sb[:, 2048:4096], d_dyT[:, 2048:4096])
        nc.sync.dma_start(eT_sb[:, 2048:4096], d_eT[:, 2048:4096])

        embT_sb = xin_sb[:, XI_EMBT:XI_EMBT + KD * T]
        utones_sb = xin_sb[:, XI_UT:XI_UT + T]
        dmaskT_sb = xin_sb[:, XI_DM:XI_DM + T]
        emb_sb = xin_sb[:, XI_EMB:XI_EMB + D]
        uts_sb = xin_sb[:, XI_UTS:XI_UTS + T]
        trik_sb = cf_sb[:, 0:T]
        xdvec_sb = cf_sb[:, T:T + 1]

        # ---- HAM warmup: dense junk matmuls while the first DMA lands.
        # All warmup MMs share ONE psum tile: PE-internal WAW needs no
        # semaphores, and every extra tile-semaphore costs ~115ns in the
        # kernel-tail drain chain on the Tensor sequencer.
        wu_ps = p_med.tile([128, 512], f32, tag="med")
        for i in range(WARMUP_MMS):
            nc.tensor.matmul(wu_ps[:], wu_sb[:, 0:128], wu_sb[:], start=True,
                             stop=True)

        def ka(n):
            # PE keepalive: the HAM re-throttles to 1.2 GHz unless the PE
            # array stays busy. 512-col junk matmuls give ~80% array duty
            # (128-col ones are half-eaten by the serialized LDWEIGHTS and
            # measured too weak to hold the clock). Per-call pool tiles
            # chain them into the real pipeline so the scheduler keeps
            # them in place (at ~115ns/tile drain-chain cost each).
            for i in range(n):
                kt = p_u.tile([128, 512], f32, tag="pu")
                nc.tensor.matmul(kt[:], wu_sb[:, 0:128], wu_sb[:],
                                 start=True, stop=True)

        # ---- u = relu(emb @ Dx.T): t-major, row sums fused -------------
        u_sb = work.tile([T, N], bf16)
        su_part = stats.tile([T, NJ + 1], f32)
        for j in range(NJ):
            ps = p_u.tile([128, 512], f32, tag="pu")
            for k in range(KD):
                c0 = XI_DX + j * 1024 + k * 512
                nc.tensor.matmul(
                    ps[:], embT_sb[:, k * T:(k + 1) * T], xin_sb[:, c0:c0 + 512],
                    start=(k == 0), stop=(k == KD - 1),
                )
            if j < NJ - 1:
                nc.vector.tensor_scalar(
                    u_sb[:, j * 512:(j + 1) * 512], ps[:], 0.0, 0.0,
                    op0=ALU.max, op1=ALU.add, accum_out=su_part[:, j:j + 1],
                )
            else:
                # split the last evac across DVE+ACT: su gates the C-chain
                # (ACT gets fewer cols — it pays an extra READ_ACCUMULATOR)
                nc.vector.tensor_scalar(
                    u_sb[:, j * 512:j * 512 + 320], ps[:, 0:320], 0.0, 0.0,
                    op0=ALU.max, op1=ALU.add, accum_out=su_part[:, j:j + 1],
                )
                nc.scalar.activation(
                    u_sb[:, j * 512 + 320:(j + 1) * 512], ps[:, 320:512],
                    AF.Relu, accum_out=su_part[:, j + 1:j + 2],
                )
            ka(2)

        # ---- C^T coefficient matrix -------------------------------------
        su = stats.tile([T, 1], f32)
        nc.vector.tensor_reduce(su[:], su_part[:], axis=AX.X, op=ALU.add)
        ka(2)  # runs while su/q compute; the real MMs below then go gapless
        q_sb = stats.tile([T, 1], bf16)
        nc.scalar.activation(q_sb[:], su[:], AF.Ln, scale=C2, bias=xdvec_sb[:])

        # exp(-Q_t) is FACTORED OUT of C^T's columns: C^T = C1 diag(e^-Q).
        # The column scale rides through x/a*/y/vraw as a positive row
        # scale (vn is pre-scaled by e^-Q_s for the contraction) and is
        # restored exactly in the final LN compensation. This removes the
        # Q-row matmul, its SBUF copy, and the broadcast matmul from the
        # critical chain.
        colsc = p_nt.tile([T, T], f32, tag="nt")            # Q_s - q_s column
        nc.tensor.matmul(colsc[:, 0:1], uts_sb[:], q_sb[:], start=True, stop=True)
        ka(3)

        expo = work.tile([T, T], f32)
        nc.vector.tensor_scalar(expo[:], trik_sb[:], colsc[:, 0:1], None,
                                op0=ALU.add)
        # no clip: trik's -1e4 pad underflows to exp->0 on the ACT spline
        ct_sb = work.tile([T, T], bf16)                     # C^T [s,t]
        nc.scalar.activation(ct_sb[:], expo[:], AF.Exp)
        ka(3)
        # Q helpers (vn pre-scale + tail compensation only) AFTER ct so
        # the eq Exp can't delay the critical ct Exp on the ACT queue
        q32 = stats.tile([T, 1], f32)
        nc.vector.tensor_copy(q32[:], q_sb[:])
        qcol = stats.tile([T, 1], f32)                      # Q_s (inclusive)
        nc.vector.scalar_tensor_tensor(qcol[:], colsc[:, 0:1], 1.0, q32[:],
                                       op0=ALU.mult, op1=ALU.add)
        eq = stats.tile([T, 1], f32)                        # e^{-Q_s}
        nc.scalar.activation(eq[:], qcol[:], AF.Exp, scale=-1.0)

        # ---- vn = LN(emb), with fused row sums (Tile slots the DVE/ACT
        # ops into engine gaps; vn is needed only after G).
        vn_sb = work.tile([T, D], bf16)
        vnsum_f = stats.tile([T, 1], f32)
        stat6 = stats.tile([T, 6], f32, tag="vn_s6")
        nc.vector.bn_stats(stat6[:], emb_sb[:])
        mv = stats.tile([T, 2], f32, tag="vn_mv")
        nc.vector.bn_aggr(mv[:], stat6[:])
        veps = stats.tile([T, 1], f32, tag="vn_ve")
        nc.vector.tensor_scalar_add(veps[:], mv[:, 1:2], LN_EPS)
        lv = stats.tile([T, 1], f32, tag="vn_lv")
        nc.scalar.activation(lv[:], veps[:], AF.Ln)
        rstd = stats.tile([T, 1], f32, tag="vn_rs")
        nc.scalar.activation(rstd[:], lv[:], AF.Exp, scale=-0.5)
        nmr = stats.tile([T, 1], f32, tag="vn_nr")
        nc.vector.scalar_tensor_tensor(nmr[:], mv[:, 0:1], -1.0, rstd[:],
                                       op0=ALU.mult, op1=ALU.mult)
        # vn' = e^{-Q_s} . LN(emb): fold the factored column scale in here
        scale2 = stats.tile([T, 1], f32, tag="vn_s2")
        nc.vector.scalar_tensor_tensor(scale2[:], rstd[:], 1.0, eq[:],
                                       op0=ALU.mult, op1=ALU.mult)
        bias2 = stats.tile([T, 1], f32, tag="vn_b2")
        nc.vector.scalar_tensor_tensor(bias2[:], nmr[:], 1.0, eq[:],
                                       op0=ALU.mult, op1=ALU.mult)
        nc.scalar.activation(vn_sb[:], emb_sb[:], AF.Identity,
                             scale=scale2[:], bias=bias2[:], accum_out=vnsum_f[:])
        vnsum_bf = stats.tile([T, 1], bf16)
        nc.vector.tensor_copy(vnsum_bf[:], vnsum_f[:])

        # ---- X^T directly n-major: X^T[n,t] = sum_s u[s,n] C^T[s,t] -----
        # ---- G = X X^T accumulated chunkwise, 2 chunks behind -----------
        xt_sb = work.tile([128, N], bf16)
        g = p_g.tile([T, T], f32, tag="g")

        def g_mm(c):
            nc.tensor.matmul(g[:], xt_sb[:, c * T:(c + 1) * T],
                             xt_sb[:, c * T:(c + 1) * T],
                             start=(c == 0), stop=(c == KN - 1))

        for c in range(KN):
            tp = p_nt.tile([128, T], f32, tag="nt")
            nc.tensor.matmul(tp[:], u_sb[:, c * 128:(c + 1) * 128], ct_sb[:],
                             start=True, stop=True)
            if c % 2 == 0:
                nc.vector.tensor_copy(xt_sb[:, c * T:(c + 1) * T], tp[:])
            else:
                nc.scalar.copy(xt_sb[:, c * T:(c + 1) * T], tp[:])
            if c >= 2:
                g_mm(c - 2)
        g_mm(KN - 2)
        g_mm(KN - 1)

        # ---- W^T = G . dmaskT; negm_row = -(W @ vnsum)/D ----------------
        wt_sb = work.tile([T, T], bf16)
        nc.vector.tensor_mul(wt_sb[:], g[:], dmaskT_sb[:])
        mp = p_nt.tile([T, T], f32, tag="nt")
        nc.tensor.matmul(mp[0:1, :], vnsum_bf[:], wt_sb[:], start=True, stop=True)
        ka(2)
        negm_row = stats.tile([1, T], bf16)
        nc.vector.tensor_scalar(negm_row[:], mp[0:1, :], -1.0 / D, None,
                                op0=ALU.mult)

        # ---- lnaT directly: a*^T chunk = vn-chunk.T @ W^T, then rank-1
        # (ones x negm_row) accumulate subtracts the row mean. The skipped
        # 1/sqrt(var_a+eps) scale is restored in the final LN.
        lnaT_sb = work.tile([128, KD * T], bf16)
        for k in range(KD):
            tp = p_nt.tile([128, T], f32, tag="nt")
            nc.tensor.matmul(tp[:], vn_sb[:, k * 128:(k + 1) * 128], wt_sb[:],
                             start=True, stop=False)
            nc.tensor.matmul(tp[:], ones1_sb[:, 0:128], negm_row[:],
                             start=False, stop=True)
            nc.vector.tensor_copy(lnaT_sb[:, k * T:(k + 1) * T], tp[:])

        # ---- a* t-major for var_a only (stats run during Yc phase) ------
        aps = p_med.tile([T, D], f32, tag="med")
        nc.tensor.matmul(aps[:], wt_sb[:], vn_sb[:], start=True, stop=True)
        stat6a = stats.tile([T, 6], f32, tag="la_s6")
        nc.vector.bn_stats(stat6a[:], aps[:])
        mv_a = stats.tile([T, 2], f32, tag="la_mv")
        nc.vector.bn_aggr(mv_a[:], stat6a[:])
        # restore the factored row scale: eps/sigma^2 =
        # eps*e^{2Q}*(var_a1 + eps*e^{2Q})
        eq2 = stats.tile([T, 1], f32)
        nc.scalar.activation(eq2[:], qcol[:], AF.Exp, scale=2.0)
        t1v = stats.tile([T, 1], f32)
        nc.vector.scalar_tensor_tensor(t1v[:], eq2[:], LN_EPS, mv_a[:, 1:2],
                                       op0=ALU.mult, op1=ALU.add)
        va_eps = stats.tile([T, 1], f32)
        nc.vector.scalar_tensor_tensor(va_eps[:], t1v[:], LN_EPS, eq2[:],
                                       op0=ALU.mult, op1=ALU.mult)

        # ---- Ycore^T n-major; Y^T = relu(Yc^T)*X^T; vraw 4 behind -------
        yt_sb = work.tile([128, N], bf16)
        vps = p_med.tile([T, D], f32, tag="med")

        def v_mm(c):
            nc.tensor.matmul(vps[:], yt_sb[:, c * T:(c + 1) * T],
                             eT_sb[:, c * D:(c + 1) * D],
                             start=(c == 0), stop=(c == KN - 1))

        for c in range(KN):
            tp = p_nt.tile([128, T], f32, tag="nt")
            for k in range(KD):
                blk = (c * KD + k) * 128
                nc.tensor.matmul(tp[:], dyT_sb[:, blk:blk + 128],
                                 lnaT_sb[:, k * T:(k + 1) * T],
                                 start=(k == 0), stop=(k == KD - 1))
            nc.vector.scalar_tensor_tensor(
                yt_sb[:, c * T:(c + 1) * T], tp[:], 0.0,
                xt_sb[:, c * T:(c + 1) * T], op0=ALU.max, op1=ALU.mult,
            )
            if c >= 3:
                v_mm(c - 3)
        for c in range(KN - 3, KN):
            v_mm(c)

        # ---- final LN with exact r_t compensation, split apply+DMA ------
        stat6v = stats.tile([T, 6], f32, tag="vs_s6")
        nc.vector.bn_stats(stat6v[:], vps[:])
        mv_v = stats.tile([T, 2], f32, tag="vs_mv")
        nc.vector.bn_aggr(mv_v[:], stat6v[:])
        veps2 = stats.tile([T, 1], f32, tag="vs_ve")
        nc.vector.scalar_tensor_tensor(veps2[:], mv_v[:, 1:2], 1.0, va_eps[:],
                                       op0=ALU.mult, op1=ALU.add)
        lv2 = stats.tile([T, 1], f32, tag="vs_lv")
        nc.scalar.activation(lv2[:], veps2[:], AF.Ln)
        rstd2 = stats.tile([T, 1], f32, tag="vs_rs")
        nc.scalar.activation(rstd2[:], lv2[:], AF.Exp, scale=-0.5)
        nmr2 = stats.tile([T, 1], f32, tag="vs_nr")
        nc.vector.scalar_tensor_tensor(nmr2[:], mv_v[:, 0:1], -1.0, rstd2[:],
                                       op0=ALU.mult, op1=ALU.mult)
        vstar_sb = work.tile([T, D], f32)
        # halves applied on DVE and ACT in parallel, each DMA'd as ready
        # (finer splits lose: each DIRECT2D costs ~600ns of issue time)
        nc.vector.tensor_scalar(vstar_sb[:, 0:128], vps[:, 0:128],
                                rstd2[:], nmr2[:], op0=ALU.mult, op1=ALU.add)
        nc.sync.dma_start(d_out[:, 0:128], vstar_sb[:, 0:128])
        nc.scalar.activation(vstar_sb[:, 128:256], vps[:, 128:256],
                             AF.Identity, scale=rstd2[:], bias=nmr2[:])
        nc.sync.dma_start(d_out[:, 128:256], vstar_sb[:, 128:256])

    return _split_multiwait(nc, mybir)


def _numpy_fallback(embeddings, E, Dx, Dy, x_state, rho_state):
    # General-path reference (only used if initial states are nonzero).
    def ln(x):
        m = x.mean(-1, keepdims=True)
        v = ((x - m) ** 2).mean(-1, keepdims=True)
        return (x - m) / np.sqrt(v + LN_EPS)

    x_s = x_state.astype(np.float32).copy()
    rho = rho_state.astype(np.float32).copy()
    outs = np.zeros((B, T, D), dtype=np.float32)
    for t in range(T):
        v_prev = embeddings[:, t, :]
        x_upd = np.maximum(v_prev @ Dx.T, 0.0)
        x_t = XD * x_s + x_upd
        x_t = x_t / np.maximum(np.abs(x_t).sum(-1, keepdims=True), L1_EPS)
        a_star = np.einsum("bdn,bn->bd", rho, x_t)
        y_core = ln(a_star) @ Dy.T
        y_t = np.maximum(y_core, 0.0) * np.maximum(x_t, 0.0)
        outs[:, t, :] = ln(y_t @ E.T)
        vn = ln(v_prev)
        rho = UD * rho + np.einsum("bd,bn->bdn", vn, x_t)
        x_s = x_t
    return outs


def _pack_inputs(embeddings, E, Dx, Dy):
    import ml_dtypes

    bf = ml_dtypes.bfloat16
    utones, dmaskT, uts = _consts_bf()
    dxT = _pack_jk(Dx.T.reshape(KD, 128, N))
    dyT = np.ascontiguousarray(
        Dy.T.reshape(KD, 128, KN, 128).transpose(1, 2, 0, 3).reshape(128, KD * N)
    ).astype(bf)
    eT = np.ascontiguousarray(
        E.T.reshape(KN, 128, D).transpose(1, 0, 2).reshape(128, KN * D)).astype(bf)
    cf = _consts_f32()

    in_maps = []
    for b in range(B):
        emb_b = embeddings[b]
        embT_b = np.ascontiguousarray(
            emb_b.T.reshape(KD, 128, T).transpose(1, 0, 2).reshape(128, KD * T))
        xin = np.concatenate([embT_b, utones, dmaskT, dxT, emb_b, uts], axis=1)
        in_maps.append({
            "xin": np.ascontiguousarray(xin).astype(bf),
            "cf": cf,
            "dyT": dyT,
            "eT": eT,
        })
    return in_maps


def kernel(embeddings, E, Dx, Dy, x_state, rho_state):
    embeddings = np.ascontiguousarray(embeddings, dtype=np.float32)
    E = np.ascontiguousarray(E, dtype=np.float32)
    Dx = np.ascontiguousarray(Dx, dtype=np.float32)
    Dy = np.ascontiguousarray(Dy, dtype=np.float32)

    if np.any(x_state) or np.any(rho_state):
        return _numpy_fallback(embeddings, E, Dx, Dy,
                               np.asarray(x_state, np.float32),
                               np.asarray(rho_state, np.float32))

    from concourse.bass_utils import run_bass_kernel_spmd

    if "nc" not in _cache:
        _cache["nc"] = _build()
    nc = _cache["nc"]

    in_maps = _pack_inputs(embeddings, E, Dx, Dy)
    res = run_bass_kernel_spmd(nc, in_maps, list(range(B)))
    _cache["last_results"] = res
    return np.stack([res.results[i]["out"].astype(np.float32) for i in range(B)])
